# revision 7
# baseline (speedup 1.0000x reference)
"""Trainium2 Bass kernel for CoOccurWithNorm.

Computes per-(image,channel) soft co-occurrence histograms of horizontally
adjacent pixel pairs, normalized by the per-histogram max.

Input  X: [64, 3, 512, 512] fp32, values in [0, 255)
Output:   [64, 3, 256, 256] fp32

Sharding: data-parallel over batch. Core k handles images [8k, 8k+8) ->
24 (image,channel) histograms per core. No cross-core communication.

Algorithm per (b,c):
  hist = sum_c W_c^T @ W_{c+1} over image columns c, accumulated in PSUM,
  where W_c is the [128 rows, 256 bins] soft one-hot (raised-cosine weights
  w0 = (1+cos(pi*f))/2 at bin ix=floor(x), w1 = 1-w0 at ix+1) of column c of
  a 128-row block. Each column's one-hot serves as rhs for chunk c-1 and
  lhsT for chunk c. One-hots are built sparsely by GPSIMD local_scatter
  (2 writes per sample) from precomputed weight/index pair streams.
  Normalization: hist / max(hist) on-device.
"""

import sys
import types
import numpy as np

sys.path.insert(0, "/root/.axon_site/_ro/trn_rl_repo")

import concourse.bass as bass
import concourse.bacc as bacc
import concourse.tile as tile
import concourse.mybir as mybir

N_CORES = 8
NBINS = 256
H = 512
W = 512
PB = 128  # partition block (rows per block)
GRP = 4  # columns per local_scatter group

_PI = float(np.pi)


def install_ntff_hook():
    """Register the axon NTFF profiling hook (missing antenv.axon_hooks shim)."""
    import antenv

    if "antenv.axon_hooks" in sys.modules:
        return
    hooks_mod = types.ModuleType("antenv.axon_hooks")
    _hook = [None]
    hooks_mod.set_axon_ntff_profile_hook = lambda h: _hook.__setitem__(0, h)
    hooks_mod.get_axon_ntff_profile_hook = lambda: _hook[0]
    sys.modules["antenv.axon_hooks"] = hooks_mod
    antenv.axon_hooks = hooks_mod
    try:
        from trn_agent_boot.trn_boot import _ntff_profile_via_ctypes

        hooks_mod.set_axon_ntff_profile_hook(
            _ntff_profile_via_ctypes("/opt/axon/libaxon_pjrt.so")
        )
    except Exception:
        pass


def build_nc(n_bc=24, n_rb=4, debug=False):
    """Build the per-core Bass module.

    n_bc: number of (image,channel) histograms this core computes.
    n_rb: number of 128-row blocks per image (4 for H=512).
    """
    f32 = mybir.dt.float32
    bf16 = mybir.dt.bfloat16
    i16 = mybir.dt.int16

    nc = bacc.Bacc("TRN2", target_bir_lowering=False, debug=debug)

    n_rows = n_bc * n_rb * PB
    XS = nc.dram_tensor("XS", [n_rows, W], f32, kind="ExternalInput")
    OUT = nc.dram_tensor("OUT", [n_bc * NBINS, NBINS], f32, kind="ExternalOutput")

    n_grp = W // GRP  # scatter groups per row block

    with tile.TileContext(nc) as tc:
        with (
            tc.tile_pool(name="const", bufs=1) as const_pool,
            tc.tile_pool(name="xin", bufs=2) as xin_pool,
            tc.tile_pool(name="bld", bufs=2) as bld_pool,
            tc.tile_pool(name="wi", bufs=2) as wi_pool,
            tc.tile_pool(name="wt", bufs=3) as wt_pool,
            tc.tile_pool(name="ep", bufs=2) as ep_pool,
            tc.tile_pool(name="psum", bufs=2, space=bass.MemorySpace.PSUM) as psum_pool,
        ):
            # Constant index-offset pattern: for column-pair slot j in [0,1024):
            #   P[j] = 256*((j//2) % GRP) + (j % 2)
            ptile = const_pool.tile([128, 2 * W], i16)
            p4 = ptile[:].rearrange("p (a b t) -> p a b t", b=GRP, t=2)
            nc.gpsimd.iota(
                p4, pattern=[[0, W // GRP], [NBINS, GRP], [1, 2]], base=0,
                channel_multiplier=0,
            )
            # bias constant for the Sin activation: cos(pi*f) = -sin(pi*f - pi/2),
            # keeping the Sin argument within the ScalarE table range [-pi, pi]
            sin_bias = const_pool.tile([128, 1], f32)
            nc.vector.memset(sin_bias[:], -_PI / 2.0)

            with tc.For_i(0, n_bc, 1) as iv:
                epsum = [
                    psum_pool.tile([128, NBINS], f32, tag="eps0", name="eps0"),
                    psum_pool.tile([128, NBINS], f32, tag="eps1", name="eps1"),
                ]
                for rb in range(n_rb):
                    # ---- load one 128-row block of the image ----
                    xt = xin_pool.tile([128, W], f32)
                    nc.sync.dma_start(
                        xt[:], XS[bass.ds(iv * (n_rb * PB) + rb * PB, PB), :]
                    )
                    # ---- build weight + index pair streams ----
                    xc = bld_pool.tile([128, W], f32, tag="xc")
                    nc.vector.tensor_scalar(xc[:], xt[:], 254.999985, None,
                                            op0=mybir.AluOpType.min)
                    # floor/frac without `mod` (not a HW TensorScalar op):
                    # rn = round-to-nearest via the 2^23 magic number, then
                    # correct rn(x) > x cases to get floor exactly.
                    rn = bld_pool.tile([128, W], f32, tag="rn")
                    nc.vector.tensor_scalar(
                        rn[:], xc[:], 8388608.0, 8388608.0,
                        op0=mybir.AluOpType.add, op1=mybir.AluOpType.subtract,
                    )
                    fr0 = bld_pool.tile([128, W], f32, tag="fr0")
                    nc.vector.tensor_sub(fr0[:], xc[:], rn[:])
                    neg = bld_pool.tile([128, W], f32, tag="neg")
                    nc.vector.tensor_scalar(neg[:], fr0[:], 0.0, None,
                                            op0=mybir.AluOpType.is_lt)
                    fr = bld_pool.tile([128, W], f32, tag="fr")
                    nc.vector.tensor_add(fr[:], fr0[:], neg[:])
                    ixf = bld_pool.tile([128, W], f32, tag="ixf")
                    nc.vector.tensor_sub(ixf[:], xc[:], fr[:])
                    cosv = bld_pool.tile([128, W], f32, tag="cosv")
                    nc.scalar.activation(
                        cosv[:], fr[:], mybir.ActivationFunctionType.Sin,
                        bias=sin_bias[:], scale=_PI,
                    )
                    # interleaved (w0, w1) bf16 pairs
                    w01 = wi_pool.tile([128, 2 * W], bf16, tag="w01")
                    nc.vector.tensor_scalar(
                        w01[:, 0 : 2 * W : 2], cosv[:], -0.5, 0.5,
                        op0=mybir.AluOpType.mult, op1=mybir.AluOpType.add,
                    )
                    nc.vector.tensor_scalar(
                        w01[:, 1 : 2 * W : 2], cosv[:], 0.5, 0.5,
                        op0=mybir.AluOpType.mult, op1=mybir.AluOpType.add,
                    )
                    # interleaved (ix, ix) int16 pairs, then += P pattern
                    idx01 = wi_pool.tile([128, 2 * W], i16, tag="idx01")
                    nc.vector.tensor_scalar(
                        idx01[:, 0 : 2 * W : 2], ixf[:], 0.0, None,
                        op0=mybir.AluOpType.add,
                    )
                    nc.vector.tensor_scalar(
                        idx01[:, 1 : 2 * W : 2], ixf[:], 0.0, None,
                        op0=mybir.AluOpType.add,
                    )
                    nc.vector.tensor_tensor(
                        idx01[:], idx01[:], ptile[:], op=mybir.AluOpType.add
                    )

                    # ---- scatter one-hots and accumulate matmuls ----
                    wts = [None] * n_grp
                    first = rb == 0
                    last = rb == n_rb - 1

                    def chunks_of(g, wts=wts, first=first, last=last):
                        # matmul chunks whose lhsT lives in group g
                        for cc in range(GRP):
                            c = GRP * g + cc
                            if c >= W - 1:
                                break
                            sub = cc
                            g2, sub2 = divmod(c + 1, GRP)
                            g2, sub2 = (c + 1) // GRP, (c + 1) % GRP
                            lhs_t = wts[g]
                            rhs_t = wts[g2]
                            st = first and c == 0
                            sp = last and c == W - 2
                            for h in range(2):
                                nc.tensor.matmul(
                                    epsum[h][:],
                                    lhs_t[:, sub * NBINS + h * 128 : sub * NBINS + h * 128 + 128],
                                    rhs_t[:, sub2 * NBINS : (sub2 + 1) * NBINS],
                                    start=st, stop=sp,
                                )

                    for g in range(n_grp):
                        wt = wt_pool.tile([128, GRP * NBINS], bf16, tag="wt")
                        nc.gpsimd.local_scatter(
                            wt[:],
                            w01[:, 2 * GRP * g : 2 * GRP * (g + 1)],
                            idx01[:, 2 * GRP * g : 2 * GRP * (g + 1)],
                            channels=128,
                            num_elems=GRP * NBINS,
                            num_idxs=2 * GRP,
                        )
                        wts[g] = wt
                        if g > 0:
                            chunks_of(g - 1)
                    chunks_of(n_grp - 1)

                # ---- epilogue: normalize by max and store ----
                mx = ep_pool.tile([128, 2], f32, tag="mx")
                for h in range(2):
                    nc.vector.tensor_reduce(
                        mx[:, h : h + 1], epsum[h][:],
                        axis=mybir.AxisListType.X, op=mybir.AluOpType.max,
                    )
                mc = ep_pool.tile([1, 2], f32, tag="mc")
                nc.gpsimd.tensor_reduce(
                    mc[:], mx[:], axis=mybir.AxisListType.C, op=mybir.AluOpType.max
                )
                vm = ep_pool.tile([1, 1], f32, tag="vm")
                nc.vector.tensor_reduce(
                    vm[:], mc[:], axis=mybir.AxisListType.X, op=mybir.AluOpType.max
                )
                rv = ep_pool.tile([1, 1], f32, tag="rv")
                nc.vector.reciprocal(rv[:], vm[:])
                rv128 = ep_pool.tile([128, 1], f32, tag="rv128")
                nc.gpsimd.partition_broadcast(rv128[:], rv[:])
                outs = ep_pool.tile([128, 2 * NBINS], f32, tag="outs")
                for h in range(2):
                    nc.vector.tensor_scalar(
                        outs[:, h * NBINS : (h + 1) * NBINS], epsum[h][:],
                        rv128[:], None, op0=mybir.AluOpType.mult,
                    )
                    nc.sync.dma_start(
                        OUT[bass.ds(iv * NBINS + h * 128, 128), :],
                        outs[:, h * NBINS : (h + 1) * NBINS],
                    )

    nc.compile()
    return nc


_NC_CACHE = {}


def _get_nc(key=(24, 4)):
    if key not in _NC_CACHE:
        _NC_CACHE[key] = build_nc(n_bc=key[0], n_rb=key[1], debug=False)
    return _NC_CACHE[key]


def kernel(X: np.ndarray) -> np.ndarray:
    """X: [64, 3, 512, 512] fp32 -> [64, 3, 256, 256] fp32."""
    from concourse.bass_utils import run_bass_kernel_spmd

    B, C, Hh, Ww = X.shape
    assert (Hh, Ww) == (H, W)
    per = B // N_CORES  # images per core
    n_bc = per * C

    nc = _get_nc((n_bc, H // PB))

    in_maps = []
    for k in range(N_CORES):
        shard = X[k * per : (k + 1) * per]  # [per, C, H, W]
        in_maps.append(
            {"XS": np.ascontiguousarray(shard.reshape(n_bc * H, W), dtype=np.float32)}
        )

    res = run_bass_kernel_spmd(nc, in_maps, core_ids=list(range(N_CORES)))
    out = np.empty((B, C, NBINS, NBINS), dtype=np.float32)
    for k in range(N_CORES):
        out[k * per : (k + 1) * per] = res.results[k]["OUT"].reshape(
            per, C, NBINS, NBINS
        )
    return out


# revision 10
# speedup vs baseline: 1.0880x; 1.0880x over previous
"""Trainium2 Bass kernel for CoOccurWithNorm.

Computes per-(image,channel) soft co-occurrence histograms of horizontally
adjacent pixel pairs, normalized by the per-histogram max.

Input  X: [64, 3, 512, 512] fp32, values in [0, 255)
Output:   [64, 3, 256, 256] fp32

Sharding: data-parallel over batch. Core k handles images [8k, 8k+8) ->
24 (image,channel) histograms per core. No cross-core communication.

Algorithm per (b,c):
  hist = sum_c W_c^T @ W_{c+1} over image columns c, accumulated in PSUM,
  where W_c is the [128 rows, 256 bins] soft one-hot (raised-cosine weights
  w0 = (1+cos(pi*f))/2 at bin ix=floor(x), w1 = 1-w0 at ix+1) of column c of
  a 128-row block. Each column's one-hot serves as rhs for chunk c-1 and
  lhsT for chunk c. One-hots are built sparsely by GPSIMD local_scatter
  (2 writes per sample) from precomputed weight/index pair streams.
  Normalization: hist / max(hist) on-device.
"""

import sys
import types
import numpy as np

sys.path.insert(0, "/root/.axon_site/_ro/trn_rl_repo")

import concourse.bass as bass
import concourse.bacc as bacc
import concourse.tile as tile
import concourse.mybir as mybir
import concourse.bass_isa as bass_isa

N_CORES = 8
NBINS = 256
H = 512
W = 512
PB = 128  # partition block (rows per block)
GRP = 7  # columns per local_scatter group (num_elems 7*256=1792, *32 < 2^16)

_PI = float(np.pi)


def install_ntff_hook():
    """Register the axon NTFF profiling hook (missing antenv.axon_hooks shim)."""
    import antenv

    if "antenv.axon_hooks" in sys.modules:
        return
    hooks_mod = types.ModuleType("antenv.axon_hooks")
    _hook = [None]
    hooks_mod.set_axon_ntff_profile_hook = lambda h: _hook.__setitem__(0, h)
    hooks_mod.get_axon_ntff_profile_hook = lambda: _hook[0]
    sys.modules["antenv.axon_hooks"] = hooks_mod
    antenv.axon_hooks = hooks_mod
    try:
        from trn_agent_boot.trn_boot import _ntff_profile_via_ctypes

        hooks_mod.set_axon_ntff_profile_hook(
            _ntff_profile_via_ctypes("/opt/axon/libaxon_pjrt.so")
        )
    except Exception:
        pass


def build_nc(n_bc=24, n_rb=4, debug=False):
    """Build the per-core Bass module.

    n_bc: number of (image,channel) histograms this core computes.
    n_rb: number of 128-row blocks per image (4 for H=512).
    """
    f32 = mybir.dt.float32
    bf16 = mybir.dt.bfloat16
    i16 = mybir.dt.int16

    nc = bacc.Bacc("TRN2", target_bir_lowering=False, debug=debug)

    n_rows = n_bc * n_rb * PB
    XS = nc.dram_tensor("XS", [n_rows, W], f32, kind="ExternalInput")
    OUT = nc.dram_tensor("OUT", [n_bc * NBINS, NBINS], f32, kind="ExternalOutput")

    with tile.TileContext(nc) as tc:
        with (
            tc.tile_pool(name="const", bufs=1) as const_pool,
            tc.tile_pool(name="xin", bufs=2) as xin_pool,
            tc.tile_pool(name="bld", bufs=2) as bld_pool,
            tc.tile_pool(name="wi", bufs=2) as wi_pool,
            tc.tile_pool(name="wt", bufs=3) as wt_pool,
            tc.tile_pool(name="ep", bufs=2) as ep_pool,
            tc.tile_pool(name="psum", bufs=2, space=bass.MemorySpace.PSUM) as psum_pool,
        ):
            # Constant index-offset pattern: for column-pair slot j in [0,1024):
            #   P[j] = 256*((j//2) % GRP) + (j % 2)
            n_grp_full = W // GRP  # full groups of GRP columns
            tail_cols = W - n_grp_full * GRP
            n_grp = n_grp_full + (1 if tail_cols else 0)
            ptile = const_pool.tile([128, n_grp * GRP * 2], i16)
            p4 = ptile[:].rearrange("p (a b t) -> p a b t", b=GRP, t=2)
            nc.gpsimd.iota(
                p4, pattern=[[0, n_grp], [NBINS, GRP], [1, 2]], base=0,
                channel_multiplier=0,
            )
            # bias constant for the Sin activation: cos(pi*f) = -sin(pi*f - pi/2),
            # keeping the Sin argument within the ScalarE table range [-pi, pi]
            sin_bias = const_pool.tile([128, 1], f32)
            nc.vector.memset(sin_bias[:], -_PI / 2.0)

            with tc.For_i(0, n_bc, 1) as iv:
                epsum = [
                    psum_pool.tile([128, NBINS], f32, tag="eps0", name="eps0"),
                    psum_pool.tile([128, NBINS], f32, tag="eps1", name="eps1"),
                ]
                for rb in range(n_rb):
                    # ---- load one 128-row block of the image ----
                    xt = xin_pool.tile([128, W], f32)
                    nc.sync.dma_start(
                        xt[:], XS[bass.ds(iv * (n_rb * PB) + rb * PB, PB), :]
                    )
                    # ---- build weight + index pair streams ----
                    xc = bld_pool.tile([128, W], f32, tag="xc")
                    nc.vector.tensor_scalar(xc[:], xt[:], 254.999985, None,
                                            op0=mybir.AluOpType.min)
                    # floor/frac without `mod` (not a HW TensorScalar op):
                    # rn = round-to-nearest via the 2^23 magic number, then
                    # correct rn(x) > x cases to get floor exactly.
                    rn = bld_pool.tile([128, W], f32, tag="rn")
                    nc.vector.tensor_scalar(
                        rn[:], xc[:], 8388608.0, 8388608.0,
                        op0=mybir.AluOpType.add, op1=mybir.AluOpType.subtract,
                    )
                    fr0 = bld_pool.tile([128, W], f32, tag="fr0")
                    nc.vector.tensor_sub(fr0[:], xc[:], rn[:])
                    neg = bld_pool.tile([128, W], f32, tag="neg")
                    nc.vector.tensor_scalar(neg[:], fr0[:], 0.0, None,
                                            op0=mybir.AluOpType.is_lt)
                    fr = bld_pool.tile([128, W], f32, tag="fr")
                    nc.vector.tensor_add(fr[:], fr0[:], neg[:])
                    ixf = bld_pool.tile([128, W], f32, tag="ixf")
                    nc.vector.tensor_sub(ixf[:], xc[:], fr[:])
                    cosv = bld_pool.tile([128, W], f32, tag="cosv")
                    nc.scalar.activation(
                        cosv[:], fr[:], mybir.ActivationFunctionType.Sin,
                        bias=sin_bias[:], scale=_PI,
                    )
                    # interleaved (w0, w1) bf16 pairs
                    w01 = wi_pool.tile([128, 2 * W], bf16, tag="w01")
                    nc.vector.tensor_scalar(
                        w01[:, 0 : 2 * W : 2], cosv[:], -0.5, 0.5,
                        op0=mybir.AluOpType.mult, op1=mybir.AluOpType.add,
                    )
                    nc.vector.tensor_scalar(
                        w01[:, 1 : 2 * W : 2], cosv[:], 0.5, 0.5,
                        op0=mybir.AluOpType.mult, op1=mybir.AluOpType.add,
                    )
                    # interleaved (ix, ix) int16 pairs, then += P pattern
                    idx01 = wi_pool.tile([128, 2 * W], i16, tag="idx01")
                    nc.vector.tensor_scalar(
                        idx01[:, 0 : 2 * W : 2], ixf[:], 0.0, None,
                        op0=mybir.AluOpType.add,
                    )
                    nc.vector.tensor_scalar(
                        idx01[:, 1 : 2 * W : 2], ixf[:], 0.0, None,
                        op0=mybir.AluOpType.add,
                    )
                    nc.vector.tensor_tensor(
                        idx01[:], idx01[:], ptile[:, 0 : 2 * W], op=mybir.AluOpType.add
                    )

                    # ---- scatter one-hots and accumulate matmuls ----
                    wts = [None] * n_grp
                    first = rb == 0
                    last = rb == n_rb - 1

                    def chunks_of(g, wts=wts, first=first, last=last):
                        # matmul chunks whose lhsT lives in group g
                        for cc in range(GRP):
                            c = GRP * g + cc
                            if c >= W - 1:
                                break
                            sub = cc
                            g2, sub2 = divmod(c + 1, GRP)
                            g2, sub2 = (c + 1) // GRP, (c + 1) % GRP
                            lhs_t = wts[g]
                            rhs_t = wts[g2]
                            st = first and c == 0
                            sp = last and c == W - 2
                            for h in range(2):
                                nc.tensor.matmul(
                                    epsum[h][:],
                                    lhs_t[:, sub * NBINS + h * 128 : sub * NBINS + h * 128 + 128],
                                    rhs_t[:, sub2 * NBINS : (sub2 + 1) * NBINS],
                                    start=st, stop=sp,
                                )

                    for g in range(n_grp):
                        ncols = min(GRP, W - GRP * g)
                        wt = wt_pool.tile([128, GRP * NBINS], bf16, tag="wt")
                        nc.gpsimd.local_scatter(
                            wt[:],
                            w01[:, 2 * GRP * g : 2 * GRP * g + 2 * ncols],
                            idx01[:, 2 * GRP * g : 2 * GRP * g + 2 * ncols],
                            channels=128,
                            num_elems=GRP * NBINS,
                            num_idxs=2 * ncols,
                        )
                        wts[g] = wt
                        if g > 0:
                            chunks_of(g - 1)
                    chunks_of(n_grp - 1)

                # ---- epilogue: normalize by max and store ----
                mx = ep_pool.tile([128, 2], f32, tag="mx")
                for h in range(2):
                    nc.vector.tensor_reduce(
                        mx[:, h : h + 1], epsum[h][:],
                        axis=mybir.AxisListType.X, op=mybir.AluOpType.max,
                    )
                ar = ep_pool.tile([128, 2], f32, tag="ar")
                nc.gpsimd.partition_all_reduce(
                    ar[:], mx[:], channels=128, reduce_op=bass_isa.ReduceOp.max
                )
                vm128 = ep_pool.tile([128, 1], f32, tag="vm128")
                nc.vector.tensor_reduce(
                    vm128[:], ar[:], axis=mybir.AxisListType.X, op=mybir.AluOpType.max
                )
                rv128 = ep_pool.tile([128, 1], f32, tag="rv128")
                nc.vector.reciprocal(rv128[:], vm128[:])
                outs = ep_pool.tile([128, 2 * NBINS], f32, tag="outs")
                for h in range(2):
                    nc.vector.tensor_scalar(
                        outs[:, h * NBINS : (h + 1) * NBINS], epsum[h][:],
                        rv128[:], None, op0=mybir.AluOpType.mult,
                    )
                    nc.sync.dma_start(
                        OUT[bass.ds(iv * NBINS + h * 128, 128), :],
                        outs[:, h * NBINS : (h + 1) * NBINS],
                    )

    nc.compile()
    return nc


_NC_CACHE = {}


def _get_nc(key=(24, 4)):
    if key not in _NC_CACHE:
        _NC_CACHE[key] = build_nc(n_bc=key[0], n_rb=key[1], debug=False)
    return _NC_CACHE[key]


def kernel(X: np.ndarray) -> np.ndarray:
    """X: [64, 3, 512, 512] fp32 -> [64, 3, 256, 256] fp32."""
    from concourse.bass_utils import run_bass_kernel_spmd

    B, C, Hh, Ww = X.shape
    assert (Hh, Ww) == (H, W)
    per = B // N_CORES  # images per core
    n_bc = per * C

    nc = _get_nc((n_bc, H // PB))

    in_maps = []
    for k in range(N_CORES):
        shard = X[k * per : (k + 1) * per]  # [per, C, H, W]
        in_maps.append(
            {"XS": np.ascontiguousarray(shard.reshape(n_bc * H, W), dtype=np.float32)}
        )

    res = run_bass_kernel_spmd(nc, in_maps, core_ids=list(range(N_CORES)))
    out = np.empty((B, C, NBINS, NBINS), dtype=np.float32)
    for k in range(N_CORES):
        out[k * per : (k + 1) * per] = res.results[k]["OUT"].reshape(
            per, C, NBINS, NBINS
        )
    return out
